# revision 4
# baseline (speedup 1.0000x reference)
"""Trainium2 Bass kernel for nn_NearestMemorySelective (scatter_memory).

Problem (hardcoded shapes):
  N_POS=8192, N_NEG=4096, LRU=1, N_SLOTS=4, D=64, M=24576
  similarity = x[:8192] @ memory.T                       [8192, 24576]
  scores     = similarity[:, :8192] + 2.0 at (i, y[i])
  y_idx      = argmax(scores, axis=1)                    [8192] int32
  get/counts = segment mean of x rows by y_idx; vis = indicator(visible)
  mpos       = EMA update of memory[:8192] + L2 normalize
  new_memory = memory with [:8192]=mpos, [12288:16384]=x[8192:]

Sharding (8 cores):
  Phase A: shard x rows (1024/core): fp32 matmul (2-way K=64 packing via
           tile_position), PSUM->SBUF copies, row argmax via max8/max_index,
           label bonus handled analytically via dma_gather(memory[y]) + dot.
  Phase B: AllGather y_idx (32KB) via DRAM bounce.
  Phase C: shard segments (1024/core): scatter-add as one-hot GEMM (fp32r,
           K=128), EMA + normalize, write new_memory rows.
"""
import sys

if "/opt/trn_rl_repo" not in sys.path:
    sys.path.insert(0, "/opt/trn_rl_repo")

import numpy as np

N_POS, N_NEG, LRU, N_SLOTS, D = 8192, 4096, 1, 4, 64
M = N_POS + N_SLOTS * N_NEG          # 24576
NCORES = 8
RPC = N_POS // NCORES                # 1024 rows / core (phase A)
SPC = N_POS // NCORES                # 1024 segments / core (phase C)
NT = RPC // 128                      # 8 row-tiles / core
NCHUNK = M // 1024                   # 24 column chunks of 1024
NSC = N_POS // 1024                  # 8 score chunks (cols < 8192)
AUGW = D + 2                         # 66: x | count-flag | vis-flag
NXJ = N_POS // 128                   # 64 x-input tiles (phase C)
NVJ = N_NEG // 128                   # 32 visible tiles (phase C)

_CACHE = {}


def _build_real():
    """Actual builder (the one used). Separated to keep _build readable was a
    mistake; this is the full, correct builder."""
    import concourse.bacc as bacc
    import concourse.tile as tile
    from concourse import mybir

    F32 = mybir.dt.float32
    F32R = mybir.dt.float32r
    I32 = mybir.dt.int32
    U32 = mybir.dt.uint32
    I16 = mybir.dt.int16
    U8 = mybir.dt.uint8
    OP = mybir.AluOpType
    AX = mybir.AxisListType
    AF = mybir.ActivationFunctionType

    nc = bacc.Bacc("TRN2", target_bir_lowering=False, debug=False,
                   num_devices=NCORES)

    xTdup_d = nc.dram_tensor("xTdup", [128, RPC], F32, kind="ExternalInput").ap()
    memTp_d = nc.dram_tensor("memTp", [128, M], F32, kind="ExternalInput").ap()
    xrows_d = nc.dram_tensor("xrows", [128, NT, D], F32, kind="ExternalInput").ap()
    yf_d = nc.dram_tensor("yf", [128, NT], F32, kind="ExternalInput").ap()
    ywrap_d = nc.dram_tensor("ywrap", [128, RPC // 16], I16, kind="ExternalInput").ap()
    mempos_d = nc.dram_tensor("mempos", [N_POS, D], F32, kind="ExternalInput").ap()
    xaug_d = nc.dram_tensor("xaug", [N_POS, AUGW], F32, kind="ExternalInput").ap()
    visf_d = nc.dram_tensor("visf", [128, NVJ], F32, kind="ExternalInput").ap()
    iota_d = nc.dram_tensor("iota", [128, SPC], F32, kind="ExternalInput").ap()
    ident_d = nc.dram_tensor("ident", [AUGW, AUGW], F32, kind="ExternalInput").ap()
    pb_d = nc.dram_tensor("pb", [128, 4], F32, kind="ExternalInput").ap()
    memseg_d = nc.dram_tensor("memseg", [SPC, D], F32, kind="ExternalInput").ap()

    sim_d = nc.dram_tensor("sim", [RPC, M], F32, kind="ExternalOutput").ap()
    yidx_d = nc.dram_tensor("yidx", [128, NT], I32, kind="ExternalOutput").ap()
    mpos_d = nc.dram_tensor("mpos", [SPC, D], F32, kind="ExternalOutput").ap()

    ycc_in = nc.dram_tensor("ycc_in", [128, NT], F32)
    ycc_out = nc.dram_tensor("ycc_out", [NCORES * 128, NT], F32,
                             addr_space="Shared")

    with tile.TileContext(nc) as tc:
        # ================= Phase A =================
        with tc.tile_pool(name="pa", bufs=1) as pa, \
             tc.tile_pool(name="stg", bufs=2) as stgp, \
             tc.tile_pool(name="sc", bufs=1) as scp, \
             tc.tile_pool(name="psA", bufs=2, space="PSUM") as psA, \
             tc.tile_pool(name="psB", bufs=2, space="PSUM") as psB:

            xT = pa.tile([128, RPC], F32, tag="xT")
            nc.sync.dma_start(xT[:], xTdup_d[:])
            memT = pa.tile([128, M], F32, tag="memT")
            nc.sync.dma_start(memT[:64, :], memTp_d[:64, :])
            nc.sync.dma_start(memT[64:, :], memTp_d[64:, :])
            xrows = pa.tile([128, NT, D], F32, tag="xrows")
            nc.sync.dma_start(xrows[:], xrows_d[:])
            yf = pa.tile([128, NT], F32, tag="yf")
            nc.sync.dma_start(yf[:], yf_d[:])
            ywrap = pa.tile([128, RPC // 16], I16, tag="ywrap")
            nc.sync.dma_start(ywrap[:], ywrap_d[:])

            memy = pa.tile([128, NT, D], F32, tag="memy")
            nc.gpsimd.dma_gather(memy[:], mempos_d[:], ywrap[:],
                                 num_idxs=RPC, num_idxs_reg=RPC, elem_size=D)

            ysel = pa.tile([128, NT], F32, tag="ysel")

            for pr in range(NT // 2):
                tA, tB = 2 * pr, 2 * pr + 1
                lhsA = xT[:64, tA * 128:(tA + 1) * 128]
                lhsB = xT[64:, tB * 128:(tB + 1) * 128]
                scA = scp.tile([128, N_POS], F32, tag="scA")
                scB = scp.tile([128, N_POS], F32, tag="scB")

                order = list(range(NSC, NCHUNK)) + list(range(NSC))
                stA = stB = None
                for c in order:
                    puA = psA.tile([128, 1024], F32, tag="puA")
                    puB = psB.tile([128, 1024], F32, tag="puB")
                    for n in range(2):
                        cs = c * 1024 + n * 512
                        nc.tensor.matmul(puA[:, n * 512:(n + 1) * 512],
                                         lhsA, memT[:64, cs:cs + 512],
                                         start=True, stop=True,
                                         tile_position=(0, 0))
                        nc.tensor.matmul(puB[:, n * 512:(n + 1) * 512],
                                         lhsB, memT[64:, cs:cs + 512],
                                         start=True, stop=True,
                                         tile_position=(64, 0))
                    if c < NSC:
                        nc.vector.tensor_copy(scA[:, c * 1024:(c + 1) * 1024], puA[:])
                        nc.scalar.copy(scB[:, c * 1024:(c + 1) * 1024], puB[:])
                    else:
                        g = (c - NSC) % 2
                        if g == 0:
                            stA = stgp.tile([128, 2048], F32, tag="stA")
                            stB = stgp.tile([128, 2048], F32, tag="stB")
                        nc.vector.tensor_copy(stA[:, g * 1024:(g + 1) * 1024], puA[:])
                        nc.scalar.copy(stB[:, g * 1024:(g + 1) * 1024], puB[:])
                        if g == 1:
                            c0 = (c - 1) * 1024
                            nc.sync.dma_start(sim_d[tA * 128:(tA + 1) * 128,
                                                    c0:c0 + 2048], stA[:])
                            nc.sync.dma_start(sim_d[tB * 128:(tB + 1) * 128,
                                                    c0:c0 + 2048], stB[:])

                nc.sync.dma_start(sim_d[tA * 128:(tA + 1) * 128, 0:N_POS], scA[:])
                nc.sync.dma_start(sim_d[tB * 128:(tB + 1) * 128, 0:N_POS], scB[:])

                for t, sc in ((tA, scA), (tB, scB)):
                    mx8 = pa.tile([128, 8], F32, tag="mx8")
                    nc.vector.max(mx8[:], sc[:])
                    idx8 = pa.tile([128, 8], U32, tag="idx8")
                    nc.vector.max_index(idx8[:], mx8[:], sc[:])
                    idxf = pa.tile([128, 1], F32, tag="idxf")
                    nc.vector.tensor_copy(idxf[:], idx8[:, 0:1])
                    junk = pa.tile([128, D], F32, tag="junk")
                    nc.vector.tensor_tensor(out=junk[:], in0=xrows[:, t, :],
                                            in1=memy[:, t, :], op=OP.mult)
                    bon = pa.tile([128, 2], F32, tag="bon")
                    nc.vector.reduce_sum(bon[:, 0:1], junk[:], axis=AX.X)
                    nc.vector.tensor_scalar_add(bon[:, 1:2], bon[:, 0:1], 2.0)
                    msk = pa.tile([128, 1], U8, tag="msk")
                    nc.vector.tensor_tensor(out=msk[:], in0=bon[:, 1:2],
                                            in1=mx8[:, 0:1], op=OP.is_gt)
                    nc.vector.tensor_copy(ysel[:, t:t + 1], idxf[:])
                    nc.vector.copy_predicated(ysel[:, t:t + 1], msk[:],
                                              yf[:, t:t + 1])

            yi32 = pa.tile([128, NT], I32, tag="yi32")
            nc.vector.tensor_copy(yi32[:], ysel[:])
            nc.sync.dma_start(yidx_d[:], yi32[:])
            nc.sync.dma_start(ycc_in[:], ysel[:])

        # ================= Phase B =================
        nc.gpsimd.collective_compute(
            "AllGather", mybir.AluOpType.bypass,
            replica_groups=[list(range(NCORES))],
            ins=[ycc_in[:]], outs=[ycc_out[:]])

        # ================= Phase C =================
        with tc.tile_pool(name="pc", bufs=1) as pc, \
             tc.tile_pool(name="pcx", bufs=4) as pcx, \
             tc.tile_pool(name="pco", bufs=4) as pco, \
             tc.tile_pool(name="psC", bufs=1, space="PSUM") as psC, \
             tc.tile_pool(name="psT", bufs=2, space="PSUM") as psT:

            yall = pc.tile([128, NXJ], F32, tag="yall")
            nc.sync.dma_start(
                yall[:], ycc_out[:].rearrange("(P a) t -> P (a t)", a=NCORES))
            iota = pc.tile([128, SPC], F32, tag="iota")
            nc.sync.dma_start(iota[:], iota_d[:])
            visf = pc.tile([128, NVJ], F32, tag="visf")
            nc.sync.dma_start(visf[:], visf_d[:])
            identt = pc.tile([AUGW, AUGW], F32, tag="ident")
            nc.sync.dma_start(identt[:], ident_d[:])
            pb = pc.tile([128, 4], F32, tag="pb")
            nc.sync.dma_start(pb[:], pb_d[:])

            vaug_f = pc.tile([128, AUGW], F32, tag="vaug_f")
            nc.vector.memset(vaug_f[:], 0.0)
            nc.vector.memset(vaug_f[:, AUGW - 1:AUGW], 1.0)
            vaug = pc.tile([128, AUGW], F32R, tag="vaug")
            nc.vector.tensor_copy(vaug[:], vaug_f[:])

            getT = psC.tile([AUGW, SPC], F32, tag="getT")
            for j in range(NXJ + NVJ):
                if j < NXJ:
                    lhs = pcx.tile([128, AUGW], F32R, tag="xaugj")
                    nc.gpsimd.dma_start(lhs[:], xaug_d[j * 128:(j + 1) * 128, :])
                    ycol = yall[:, j:j + 1]
                else:
                    lhs = vaug
                    ycol = visf[:, j - NXJ:j - NXJ + 1]
                oh = pco.tile([128, SPC], F32R, tag="oh")
                nc.vector.tensor_scalar(out=oh[:], in0=iota[:], scalar1=ycol,
                                        scalar2=None, op0=OP.is_equal)
                for n in range(2):
                    nc.tensor.matmul(getT[:, n * 512:(n + 1) * 512], lhs[:],
                                     oh[:, n * 512:(n + 1) * 512],
                                     start=(j == 0), stop=(j == NXJ + NVJ - 1))

            getTs = pc.tile([AUGW, SPC], F32, tag="getTs")
            nc.vector.tensor_copy(getTs[:], getT[:])

            one_m = pc.tile([128, 1], F32, tag="one_m")
            nc.vector.tensor_scalar(out=one_m[:], in0=pb[:, 3:4], scalar1=-1.0,
                                    scalar2=1.0, op0=OP.mult, op1=OP.add)

            for st in range(NT):
                ptr = psT.tile([128, AUGW], F32, tag="ptr")
                nc.tensor.transpose(ptr[:], getTs[:, st * 128:(st + 1) * 128],
                                    identt[:])
                g = pc.tile([128, AUGW], F32, tag="g")
                nc.scalar.copy(g[:], ptr[:])

                memt = pc.tile([128, D], F32, tag="memt")
                base = st * 128
                nc.sync.dma_start(memt[:], memseg_d[base:base + 128, :])

                counts = pc.tile([128, 4], F32, tag="counts")
                # counts, visnum
                nc.vector.tensor_scalar(out=counts[:, 0:1], in0=g[:, D:D + 1],
                                        scalar1=1e-8, scalar2=None, op0=OP.add)
                # validc = counts > 0.1 ; visflag = visnum > 0.5
                nc.vector.tensor_scalar(out=counts[:, 1:2], in0=g[:, D:D + 1],
                                        scalar1=0.1, scalar2=None, op0=OP.is_gt)
                nc.vector.tensor_scalar(out=counts[:, 2:3], in0=g[:, D + 1:D + 2],
                                        scalar1=0.5, scalar2=None, op0=OP.is_gt)
                valid = pc.tile([128, 3], F32, tag="valid")
                nc.vector.tensor_tensor(out=valid[:, 0:1], in0=counts[:, 1:2],
                                        in1=counts[:, 2:3], op=OP.mult)
                # coefg = valid * (1 - m); coefm = 1 - coefg
                nc.vector.tensor_tensor(out=valid[:, 1:2], in0=valid[:, 0:1],
                                        in1=one_m[:], op=OP.mult)
                nc.vector.tensor_scalar(out=valid[:, 2:3], in0=valid[:, 1:2],
                                        scalar1=-1.0, scalar2=1.0,
                                        op0=OP.mult, op1=OP.add)
                rec = pc.tile([128, 1], F32, tag="rec")
                nc.vector.reciprocal(rec[:], counts[:, 0:1])
                # gavg = get/counts * coefg   (fold into one per-partition scalar)
                gsc = pc.tile([128, 1], F32, tag="gsc")
                nc.vector.tensor_tensor(out=gsc[:], in0=rec[:],
                                        in1=valid[:, 1:2], op=OP.mult)
                t1 = pc.tile([128, D], F32, tag="t1")
                nc.vector.tensor_scalar(out=t1[:], in0=g[:, 0:D],
                                        scalar1=gsc[:], scalar2=None,
                                        op0=OP.mult)
                t2 = pc.tile([128, D], F32, tag="t2")
                nc.vector.tensor_scalar(out=t2[:], in0=memt[:],
                                        scalar1=valid[:, 2:3], scalar2=None,
                                        op0=OP.mult)
                mp = pc.tile([128, D], F32, tag="mp")
                nc.vector.tensor_tensor(out=mp[:], in0=t1[:], in1=t2[:],
                                        op=OP.add)
                # L2 normalize
                sq = pc.tile([128, D], F32, tag="sq")
                nsq = pc.tile([128, 1], F32, tag="nsq")
                nc.scalar.activation(sq[:], mp[:], AF.Square, accum_out=nsq[:])
                nrm = pc.tile([128, 1], F32, tag="nrm")
                nc.scalar.activation(nrm[:], nsq[:], AF.Sqrt)
                rn = pc.tile([128, 1], F32, tag="rn")
                nc.vector.reciprocal(rn[:], nrm[:])
                mpo = pc.tile([128, D], F32, tag="mpo")
                nc.vector.tensor_scalar(out=mpo[:], in0=mp[:], scalar1=rn[:],
                                        scalar2=None, op0=OP.mult)
                nc.sync.dma_start(mpos_d[st * 128:(st + 1) * 128, :], mpo[:])

    nc.compile()
    return nc


def _host_prep(x, y, visible, memory, params):
    x = np.asarray(x, dtype=np.float32)
    y = np.asarray(y).astype(np.int64).ravel()
    visible = np.asarray(visible).astype(np.int64).ravel()
    memory = np.asarray(memory, dtype=np.float32)
    params = np.asarray(params, dtype=np.float32).ravel()

    xp = x[:N_POS]
    xT = np.ascontiguousarray(xp.T)                       # [64, 8192]
    memT = np.ascontiguousarray(memory.T)                 # [64, 24576]
    memTp = np.concatenate([memT, memT], axis=0)          # [128, 24576]
    mempos = np.ascontiguousarray(memory[:N_POS])         # [8192, 64]

    # xp_aug (x | 1 | 0), shuffled for phase-C tiles:
    # row (j*128 + P) = xp_aug[g(P, j)],
    # g(P,j) = (P>>4)*1024 + (j&7)*128 + (P&15)*8 + (j>>3)
    xaug = np.concatenate(
        [xp, np.ones((N_POS, 1), np.float32), np.zeros((N_POS, 1), np.float32)],
        axis=1)
    J = np.arange(NXJ)
    P = np.arange(128)
    gidx = ((P[None, :] >> 4) * 1024 + (J[:, None] & 7) * 128 +
            (P[None, :] & 15) * 8 + (J[:, None] >> 3))    # [64, 128]
    xaug_shuf = np.ascontiguousarray(xaug[gidx.ravel()])  # [8192, 66]

    visf = np.ascontiguousarray(
        visible.reshape(NVJ, 128).T.astype(np.float32))   # [128, 32]
    ident = np.eye(AUGW, dtype=np.float32)
    pb = np.tile(params[None, :], (128, 1))

    per_core = []
    for k in range(NCORES):
        yk = y[k * RPC:(k + 1) * RPC]
        xTk = np.ascontiguousarray(xT[:, k * RPC:(k + 1) * RPC])
        xTdup = np.concatenate([xTk, xTk], axis=0)        # [128, 1024]
        xrows = np.ascontiguousarray(
            xp[k * RPC:(k + 1) * RPC].reshape(NT, 128, D).transpose(1, 0, 2))
        yfk = np.ascontiguousarray(
            yk.reshape(NT, 128).T.astype(np.float32))     # [128, 8]
        ywrap = np.zeros((128, RPC // 16), np.int16)
        jj = np.arange(RPC)
        for gblk in range(8):
            ywrap[jj % 16 + 16 * gblk, jj // 16] = yk
        iota = np.tile(
            (k * SPC + np.arange(SPC)).astype(np.float32), (128, 1))
        memseg = np.ascontiguousarray(memory[k * SPC:(k + 1) * SPC])
        per_core.append({
            "xTdup": xTdup, "memTp": memTp, "xrows": xrows, "yf": yfk,
            "ywrap": ywrap, "mempos": mempos, "xaug": xaug_shuf,
            "visf": visf, "iota": iota, "ident": ident, "pb": pb,
            "memseg": memseg,
        })
    return per_core


def run(inputs, trace=False):
    from concourse.bass_utils import run_bass_kernel_spmd

    if "nc" not in _CACHE:
        _CACHE["nc"] = _build_real()
    nc = _CACHE["nc"]

    x = np.asarray(inputs["x"], dtype=np.float32)
    memory = np.asarray(inputs["memory"], dtype=np.float32)
    in_maps = _host_prep(x, inputs["y"], inputs["visible"], memory,
                         inputs["params"])
    res = run_bass_kernel_spmd(nc, in_maps, core_ids=list(range(NCORES)),
                               trace=trace)

    sim = np.concatenate([r["sim"] for r in res.results], axis=0)
    yidx = np.concatenate(
        [r["yidx"].T.reshape(RPC) for r in res.results], axis=0).astype(np.int32)
    new_memory = memory.copy()
    for k, r in enumerate(res.results):
        new_memory[k * SPC:(k + 1) * SPC] = r["mpos"]
    start = N_POS + LRU * N_NEG
    new_memory[start:start + N_NEG] = x[N_POS:]
    return (sim, yidx, new_memory), res


def kernel(**inputs):
    (sim, yidx, new_memory), _ = run(inputs, trace=False)
    return sim, yidx, new_memory
